# revision 24
# baseline (speedup 1.0000x reference)
"""AFT2D attention Trainium2 kernel (8 NeuronCores, data-parallel over batch).

Math: the reference's 5x5 windowed attention with positional bias
    wgt = exp(w_h[ii]*(di-h) + w_v[jj]*(dj-w) + k[h+di, w+dj]) * mask
factorizes exactly: exp(bias) separates into per-row and per-column factors,
so with ek = exp(k), u = ek*v, s = sum_d ek:
    out  = A @ (B ∘w u)      (two banded 64x64 matrix passes, h then w)
    norm = A @ (B ∘w s)
    y    = (out @ Wp^T) / (norm + eps)      (normalize commutes past Wp)
where A[h,h'] = exp(w_h[h'-h+R]*((h'-h)-h)) on the band, B likewise over w.

v3 pipeline (b_loc=2 images; partitions = (b,w), then (h,b) after the
h<->w shuffle). Engine budget per 8h in phase A ~4.1-4.3us on each of
PE/ACT/DVE/GpSimd; phase B balanced ACT/DVE under the PE's ~0.86us/g:
  1. k|v matmul per h-pair into a 2-bank psum tile [128, 2, 512]
     (x-tiles as PE stationary, streaming [Wk^T | Wv^T]).
  2. ACT: ONE batched exp per h-pair [128, 2, 256] -> ek.
  3. DVE: u = ek * v per h-pair.  GpSimd: s = sum_d ek per h via
     tensor_scalar+accum_out (reads ek from SBUF; GpSimd has no PSUM
     port so it can't take drains).
  4. Horizontal pass per 8h: 4 matmuls vs blkdiag(B^T,B^T); drains
     alternate ACT/DVE.
  5. h<->w shuffle via HBM bounce (big batched transfers; SBUF->SBUF
     per-h DMAs lose on descriptor dispatch): BOTH legs on Sync HWDGE,
     leg 2 directly behind leg 1 in the FIFO (its sem wait overlaps the
     next block's compute; Sync dispatch is far snappier than GpSimd
     SWDGE which used to add ~5us to the A->B transition).
  6. Vertical pass, swapped operands -> feature-major psum [d-half, (b,h)]
     = the projection's lhsT; projection matmul; ot drains alternate
     ACT/DVE; y drains (scaled by 1/norm) split 50/50 ACT/DVE.
Input x DMAs are triggered round-robin from Sync/GpSimd (ACT stays free:
it paces phase A) so the first k|v matmul starts early.
Norm path runs in f32 on the side (tiny matmuls; per-b col-tiled output).
"""
import sys

sys.path.insert(0, "/opt/trn_rl_repo")

import numpy as np
import ml_dtypes

import concourse.bass as bass
import concourse.mybir as mybir
import concourse.tile as tile
from concourse.bass_utils import run_bass_kernel_spmd

bf16 = ml_dtypes.bfloat16

N_CORES = 8
B_FULL, H, W, C = 16, 64, 64, 256
D = 256   # HID
O = 256   # OUT
R = 2
B_LOC = B_FULL // N_CORES  # 2

LAST_RESULT = None
_CACHED_NC = None


def _split_multi_waits(nc, max_waits=1):
    """This container's walrus accepts at most ONE sync-wait per instruction;
    hoist extras into standalone same-engine no-ops (order-preserving)."""
    n_new = 0
    for func in nc.m.functions:
        for blk in func.blocks:
            new_insts = []
            for inst in blk.instructions:
                si = inst.sync_info
                if si is not None and len(si.on_wait) > max_waits:
                    waits = list(si.on_wait)
                    for w in waits[:-max_waits]:
                        nop = mybir.InstNoOp(
                            name=f"waitsplit-{n_new}-{inst.name}", ins=[], outs=[])
                        nop.engine = inst.engine
                        nop.sync_info = mybir.SyncInfo(on_wait=[w], on_update=[])
                        new_insts.append(nop)
                        n_new += 1
                    si.on_wait = waits[-max_waits:]
                new_insts.append(inst)
            blk.instructions = new_insts
    return n_new


# packed bf16 constant blob column offsets
_WKV_OFF = 0                 # [128, 2, 512]
_WP_OFF = _WKV_OFF + 1024    # [128, 2, 256]
_BH_OFF = _WP_OFF + 512      # [128, 128]
_AV_OFF = _BH_OFF + 128      # [128, 128]
_CB_COLS = _AV_OFF + 128
# f32 blob: bh_f32 [128,128] then a64t [64,64] (cols 128..192)
_FB_COLS = 192


def _build_nc():
    fp32 = mybir.dt.float32
    bft = mybir.dt.bfloat16
    Exp = mybir.ActivationFunctionType.Exp
    Ax = mybir.AxisListType
    Alu = mybir.AluOpType

    nc = bass.Bass()
    xt_ext = nc.declare_dram_parameter("xt", [128, H, 2, 128], bft, isOutput=False)
    cb_ext = nc.declare_dram_parameter("cblob", [128, _CB_COLS], bft, isOutput=False)
    fb_ext = nc.declare_dram_parameter("fblob", [128, _FB_COLS], fp32, isOutput=False)
    y_ext = nc.declare_dram_parameter("y", [B_LOC, H, W, O], bft, isOutput=True)
    # HBM bounce for the h<->w shuffle: layout (h, b, w, d) -- strided write,
    # contiguous read, pipelined in 8h-eighths (no global barrier)
    tbounce = nc.dram_tensor("tbounce", [H, B_LOC, W, D], bft)

    with tile.TileContext(nc) as tc:
        with (
            tc.tile_pool(name="const", bufs=1) as cpool,
            tc.tile_pool(name="ek", bufs=4) as ek_pool,
            tc.tile_pool(name="slab", bufs=1) as slab_pool,
            tc.tile_pool(name="ot", bufs=4) as ot_pool,
            tc.tile_pool(name="y", bufs=4) as y_pool,
        ):
            # ---- constants; wkv first (first matmul's dependency)
            cblob = cpool.tile([128, _CB_COLS], bft)
            fblob = cpool.tile([128, _FB_COLS], fp32)
            xt_slab = slab_pool.tile([128, H, 2, 128], bft)

            # trigger order: wkv -> x piece 0 -> rest of cblob -> staggered x
            # pieces round-robin across Sync/GpSimd (ACT kept free: it is the
            # phase-A pacing engine) -> fblob
            # first 512 cols = wkv cc-chunk 0: everything the first k|v
            # matmul needs besides x piece 0
            nc.sync.dma_start(out=cblob[:, 0:512], in_=cb_ext[:, 0:512])
            # few, staggered x pieces: small head pieces so the first k|v
            # matmul starts early, 8h bodies to stay off the semaphore pool
            xp = [(0, 2), (2, 4), (4, 8)] + [(q * 8, (q + 1) * 8)
                                             for q in range(1, 8)]

            def _xpiece(eng, q):
                a, b = xp[q]
                eng.dma_start(out=xt_slab[:, a:b], in_=xt_ext[:, a:b])

            _xpiece(nc.sync, 0)
            _xpiece(nc.gpsimd, 1)
            _xpiece(nc.gpsimd, 2)
            nc.sync.dma_start(out=cblob[:, 512:], in_=cb_ext[:, 512:])
            for q in range(3, 10):
                eng = (nc.sync, nc.gpsimd)[q % 2]
                _xpiece(eng, q)
            nc.sync.dma_start(out=fblob[:], in_=fb_ext[:])

            wkv_sb = cblob[:, _WKV_OFF:_WKV_OFF + 1024].rearrange(
                "p (c d) -> p c d", c=2)
            wp_sb = cblob[:, _WP_OFF:_WP_OFF + 512].rearrange(
                "p (c d) -> p c d", c=2)
            bh_bd = cblob[:, _BH_OFF:_BH_OFF + 128]
            av_mix = cblob[:, _AV_OFF:_AV_OFF + 128]
            bh_f32 = fblob[:, 0:128]
            a64t = fblob[0:64, 128:192]

            u_slab = slab_pool.tile([128, H, D], bft)      # [(b,w), h, d]
            t_slab = slab_pool.tile([128, H, D], bft)      # [(b,w), h, d]
            tt_slab = slab_pool.tile([128, W, D], bft)     # [(h,b)=2h+b, w, d]
            ek_slab = slab_pool.tile([128, H, D], bft)     # [(b,w), h, d]
            s_slab = slab_pool.tile([128, H], fp32)        # [(b,w), h]
            sl1 = slab_pool.tile([128, 2, 8, 128], fp32)   # s-tree lvl1 (2 bufs)
            sl2 = slab_pool.tile([128, 2, 8, 64], fp32)    # s-tree lvl2 (2 bufs)
            u_flat = u_slab.rearrange("p h d -> p (h d)")
            t_flat = t_slab.rearrange("p h d -> p (h d)")
            tt_flat = tt_slab.rearrange("p w d -> p (w d)")
            tb_w = tbounce.rearrange("h b w d -> b w h d")   # leg-1 write view
            tb_r = tbounce.rearrange("h b w d -> (h b) w d")  # leg-2 read view
            sth = cpool.tile([64, 128], fp32, tag="sth")

            with (
                tc.tile_pool(name="ps_kv", bufs=3, space="PSUM") as ps_kv_pool,
                tc.tile_pool(name="ps_t", bufs=2, space="PSUM") as ps_t_pool,
            ):
                # ---- phase A: k|v matmuls + batched exp + u-mul; s via a
                # 2-level GpSimd add-tree (SBUF-only, so the PSUM-blind
                # engine finally carries real work) + cheap DVE finisher;
                # horizontal pass and the h<->w shuffle interleaved per 8h
                for hp in range(H // 2):          # h-pair index
                    h0 = 2 * hp
                    ps2 = ps_kv_pool.tile([128, 2, 512], fp32)
                    for j in range(2):
                        nc.tensor.matmul(ps2[:, j, :], xt_slab[:, h0 + j, 0, :],
                                         wkv_sb[:, 0, :], start=True, stop=False)
                        nc.tensor.matmul(ps2[:, j, :], xt_slab[:, h0 + j, 1, :],
                                         wkv_sb[:, 1, :], start=False, stop=True)
                    ek = ek_slab[:, h0:h0 + 2, :]
                    nc.scalar.activation(ek, ps2[:, :, 0:256], Exp)
                    nc.vector.tensor_mul(u_slab[:, h0:h0 + 2, :], ek,
                                         ps2[:, :, 256:512])
                    # horizontal matmul + drain PER PAIR: its only dep is
                    # this pair's u, so the drain never waits on late PE
                    # work from a whole 8h batch (that coupling made the
                    # batched variants latency-bound)
                    ps_t = ps_t_pool.tile([128, 512], fp32)
                    nc.tensor.matmul(ps_t[:], bh_bd[:],
                                     u_flat[:, hp * 512:(hp + 1) * 512],
                                     start=True, stop=True)
                    if hp % 8 in (0, 2, 4, 6, 1):
                        nc.scalar.copy(t_flat[:, hp * 512:(hp + 1) * 512],
                                       ps_t[:])
                    else:
                        nc.vector.tensor_copy(
                            t_flat[:, hp * 512:(hp + 1) * 512], ps_t[:])

                    if hp % 4 == 3:
                        q = hp // 4
                        h8 = 8 * q
                        qb = q % 2
                        # s-tree for this 8h: two GpSimd adds halving d; the
                        # DVE finisher is deferred one block so it never
                        # heads DVE's FIFO while the tree is still running
                        ekq = ek_slab[:, h8:h8 + 8, :]
                        nc.gpsimd.tensor_add(sl1[:, qb], ekq[:, :, 0:128],
                                             ekq[:, :, 128:256])
                        nc.gpsimd.tensor_add(sl2[:, qb], sl1[:, qb, :, 0:64],
                                             sl1[:, qb, :, 64:128])
                        if q >= 1:
                            nc.vector.tensor_reduce(
                                s_slab[:, h8 - 8:h8], sl2[:, 1 - qb],
                                Ax.X, Alu.add)
                        # bounce leg 1 on Sync, alone on its FIFO: issues
                        # the moment this block's drains land.  Leg 2 of the
                        # PREVIOUS block goes on GpSimd here -- one block of
                        # deferral means its leg-1-receipt wait is already
                        # satisfied when dequeued, so nothing ever piles up
                        # at the end of phase A.
                        nc.sync.dma_start(
                            out=tb_w[:, :, h8:h8 + 8, :],
                            in_=t_slab[:, h8:h8 + 8, :])
                        if q >= 1:
                            for wl, wh in ((0, 32), (32, 64)):
                                nc.gpsimd.dma_start(
                                    out=tt_slab[16 * q - 16:16 * q, wl:wh, :],
                                    in_=tb_r[16 * q - 16:16 * q, wl:wh, :])
                # last block's s finisher + last leg 2s
                nc.vector.tensor_reduce(s_slab[:, H - 8:H], sl2[:, 1],
                                        Ax.X, Alu.add)
                for wl, wh in ((0, 32), (32, 64)):
                    nc.gpsimd.dma_start(
                        out=tt_slab[112:128, wl:wh, :],
                        in_=tb_r[112:128, wl:wh, :])

                # norm-horizontal (swapped: out comes transposed [h, (b,w)])
                ps_sh = ps_t_pool.tile([64, 128], fp32, tag="ps_t")
                nc.tensor.matmul(ps_sh[:], s_slab[:], bh_f32[:],
                                 start=True, stop=True)
                nc.vector.tensor_copy(sth[:], ps_sh[:])

            with (
                tc.tile_pool(name="ps_g", bufs=4, space="PSUM") as ps_g_pool,
                tc.tile_pool(name="ps_y", bufs=4, space="PSUM") as ps_y_pool,
            ):
                # norm-vertical: per-b matmuls into partition halves (col tiling)
                ps_n = ps_y_pool.tile([128, 64], fp32, tag="ps_y")
                for b in range(B_LOC):
                    nc.tensor.matmul(ps_n[b * 64:(b + 1) * 64, :], a64t[:],
                                     sth[:, b * 64:(b + 1) * 64],
                                     start=True, stop=True,
                                     tile_position=(0, b * 64))
                ntmp = cpool.tile([128, 64], fp32, tag="ntmp")
                nc.vector.tensor_scalar_add(ntmp[:], ps_n[:], 1e-8)
                rnorm = cpool.tile([128, 64], fp32, tag="rnorm")
                nc.vector.reciprocal(rnorm[:], ntmp[:])

                # ---- phase B: vertical pass (swapped -> feature-major) + proj
                for g in range(W // 2):           # w-pair per psum group
                    ps_g = ps_g_pool.tile([128, 512], fp32)
                    for q in range(4):
                        cch = g * 4 + q   # chunk = (w = cch>>1, dhalf = cch&1)
                        nc.tensor.matmul(ps_g[:, q * 128:(q + 1) * 128],
                                         tt_flat[:, cch * 128:(cch + 1) * 128],
                                         av_mix[:], start=True, stop=True)
                    ot = ot_pool.tile([128, 512], bft)
                    if g % 2 == 0:
                        nc.scalar.copy(ot[:], ps_g[:])
                    else:
                        nc.vector.tensor_copy(ot[:], ps_g[:])

                    for wi in range(2):
                        w = 2 * g + wi
                        base = wi * 256
                        ps_y = ps_y_pool.tile([128, O], fp32)
                        nc.tensor.matmul(ps_y[:], ot[:, base:base + 128],
                                         wp_sb[:, 0, :], start=True, stop=False)
                        nc.tensor.matmul(ps_y[:], ot[:, base + 128:base + 256],
                                         wp_sb[:, 1, :], start=False, stop=True)
                        if w % 4 == 0:
                            y4 = y_pool.tile([128, 4, O], bft)
                        yt = y4[:, w % 4, :]
                        # y drains: half DVE, half ACT (paired with the
                        # opposite engine's ot drain for this g)
                        if wi == (g % 2):
                            nc.scalar.mul(yt[:], ps_y[:], rnorm[:, w:w + 1])
                        else:
                            nc.vector.tensor_scalar_mul(yt[:], ps_y[:],
                                                        rnorm[:, w:w + 1])
                        if w % 4 == 3:
                            w0 = w - 3
                            nc.sync.dma_start(
                                out=y_ext[:, :, w0:w0 + 4, :].rearrange(
                                    "b h w o -> (b h) w o"),
                                in_=y4[:])

    _split_multi_waits(nc)
    return nc


def _host_prep(x, w_h, w_v, Wk, Wv, Wp):
    """Build per-core input maps (all layout/packing on host, compute on device)."""
    A = np.zeros((H, H), np.float32)
    Bm = np.zeros((W, W), np.float32)
    for h in range(H):
        for hp in range(max(0, h - R), min(H, h + R + 1)):
            A[h, hp] = np.exp(w_h[hp - h + R] * ((hp - h) - h))
    for w in range(W):
        for wp in range(max(0, w - R), min(W, w + R + 1)):
            Bm[w, wp] = np.exp(w_v[wp - w + R] * ((wp - w) - w))

    eye2 = np.eye(2, dtype=np.float32)
    bh_bd = np.kron(eye2, Bm.T)                      # lhsT for horizontal
    # vertical rhs, rows (h',b)-interleaved, cols (b,h) b-major:
    av_mix = np.zeros((128, 128), np.float32)
    for b in range(B_LOC):
        for h in range(H):
            for hp in range(max(0, h - R), min(H, h + R + 1)):
                av_mix[2 * hp + b, 64 * b + h] = A[h, hp]

    # wkv[ci, cc, j] = Wk[j, cc*128+ci] (j<256) else Wv[j-256, ...]
    wkv = np.concatenate([Wk.T, Wv.T], axis=1)       # [C, 2D]
    wkv = wkv.reshape(2, 128, 2 * D).transpose(1, 0, 2)      # [ci, cc, 512]
    wp = Wp.T.reshape(2, 128, O).transpose(1, 0, 2)          # [di, dc, o]

    cblob = np.empty((128, _CB_COLS), np.float32)
    cblob[:, _WKV_OFF:_WKV_OFF + 1024] = wkv.reshape(128, 1024)
    cblob[:, _WP_OFF:_WP_OFF + 512] = wp.reshape(128, 512)
    cblob[:, _BH_OFF:_BH_OFF + 128] = bh_bd
    cblob[:, _AV_OFF:_AV_OFF + 128] = av_mix
    cblob = cblob.astype(bf16)

    fblob = np.zeros((128, _FB_COLS), np.float32)
    fblob[:, 0:128] = bh_bd
    fblob[0:64, 128:192] = A.T

    in_maps = []
    for c in range(N_CORES):
        xl = x[c * B_LOC:(c + 1) * B_LOC]            # (2, 64, 64, 256)
        t = xl.reshape(B_LOC, H, W, 2, 128)          # b h w cc ci
        xt = np.ascontiguousarray(
            t.transpose(4, 1, 3, 0, 2).reshape(128, H, 2, 128)).astype(bf16)
        in_maps.append({"xt": xt, "cblob": cblob, "fblob": fblob})
    return in_maps


def kernel(x, w_h, w_v, Wk, Wv, Wp):
    global LAST_RESULT, _CACHED_NC
    x = np.asarray(x, dtype=np.float32)
    w_h = np.asarray(w_h, dtype=np.float32)
    w_v = np.asarray(w_v, dtype=np.float32)
    Wk = np.asarray(Wk, dtype=np.float32)
    Wv = np.asarray(Wv, dtype=np.float32)
    Wp = np.asarray(Wp, dtype=np.float32)

    in_maps = _host_prep(x, w_h, w_v, Wk, Wv, Wp)
    if _CACHED_NC is None:
        _CACHED_NC = _build_nc()
    res = run_bass_kernel_spmd(_CACHED_NC, in_maps, core_ids=list(range(N_CORES)))
    LAST_RESULT = res

    out = np.empty((B_FULL, H, W, O), np.float32)
    for c in range(N_CORES):
        yc = np.asarray(res.results[c]["y"]).astype(np.float32)
        out[c * B_LOC:(c + 1) * B_LOC] = yc
    return out



# revision 27
# speedup vs baseline: 1.1151x; 1.1151x over previous
"""AFT2D attention Trainium2 kernel (8 NeuronCores, data-parallel over batch).

Math: the reference's 5x5 windowed attention with positional bias
    wgt = exp(w_h[ii]*(di-h) + w_v[jj]*(dj-w) + k[h+di, w+dj]) * mask
factorizes exactly: exp(bias) separates into per-row and per-column factors,
so with ek = exp(k), u = ek*v, s = sum_d ek:
    out  = A @ (B ∘w u)      (two banded 64x64 matrix passes, h then w)
    norm = A @ (B ∘w s)
    y    = (out @ Wp^T) / (norm + eps)      (normalize commutes past Wp)
where A[h,h'] = exp(w_h[h'-h+R]*((h'-h)-h)) on the band, B likewise over w.

v3 pipeline (b_loc=2 images; partitions = (b,w), then (h,b) after the
h<->w shuffle). Engine budget per 8h in phase A ~4.1-4.3us on each of
PE/ACT/DVE/GpSimd; phase B balanced ACT/DVE under the PE's ~0.86us/g:
  1. k|v matmul per h-pair into a 2-bank psum tile [128, 2, 512]
     (x-tiles as PE stationary, streaming [Wk^T | Wv^T]).
  2. ACT: ONE batched exp per h-pair [128, 2, 256] -> ek.
  3. DVE: u = ek * v per h-pair.  GpSimd: s = sum_d ek per h via
     tensor_scalar+accum_out (reads ek from SBUF; GpSimd has no PSUM
     port so it can't take drains).
  4. Horizontal pass per 8h: 4 matmuls vs blkdiag(B^T,B^T); drains
     alternate ACT/DVE.
  5. h<->w shuffle via HBM bounce (big batched transfers; SBUF->SBUF
     per-h DMAs lose on descriptor dispatch): BOTH legs on Sync HWDGE,
     leg 2 directly behind leg 1 in the FIFO (its sem wait overlaps the
     next block's compute; Sync dispatch is far snappier than GpSimd
     SWDGE which used to add ~5us to the A->B transition).
  6. Vertical pass, swapped operands -> feature-major psum [d-half, (b,h)]
     = the projection's lhsT; projection matmul; ot drains alternate
     ACT/DVE; y drains (scaled by 1/norm) split 50/50 ACT/DVE.
Input x DMAs are triggered round-robin from Sync/GpSimd (ACT stays free:
it paces phase A) so the first k|v matmul starts early.
Norm path runs in f32 on the side (tiny matmuls; per-b col-tiled output).
"""
import sys

sys.path.insert(0, "/opt/trn_rl_repo")

import numpy as np
import ml_dtypes

import concourse.bass as bass
import concourse.mybir as mybir
import concourse.tile as tile
from concourse.bass_utils import run_bass_kernel_spmd

bf16 = ml_dtypes.bfloat16

N_CORES = 8
B_FULL, H, W, C = 16, 64, 64, 256
D = 256   # HID
O = 256   # OUT
R = 2
B_LOC = B_FULL // N_CORES  # 2

LAST_RESULT = None
_CACHED_NC = None


def _split_multi_waits(nc, max_waits=1):
    """This container's walrus accepts at most ONE sync-wait per instruction;
    hoist extras into standalone same-engine no-ops (order-preserving)."""
    n_new = 0
    for func in nc.m.functions:
        for blk in func.blocks:
            new_insts = []
            for inst in blk.instructions:
                si = inst.sync_info
                if si is not None and len(si.on_wait) > max_waits:
                    waits = list(si.on_wait)
                    for w in waits[:-max_waits]:
                        nop = mybir.InstNoOp(
                            name=f"waitsplit-{n_new}-{inst.name}", ins=[], outs=[])
                        nop.engine = inst.engine
                        nop.sync_info = mybir.SyncInfo(on_wait=[w], on_update=[])
                        new_insts.append(nop)
                        n_new += 1
                    si.on_wait = waits[-max_waits:]
                new_insts.append(inst)
            blk.instructions = new_insts
    return n_new


# packed bf16 constant blob column offsets
_WKV_OFF = 0                 # [128, 2, 512]
_WP_OFF = _WKV_OFF + 1024    # [128, 2, 256]
_BH_OFF = _WP_OFF + 512      # [128, 128]
_AV_OFF = _BH_OFF + 128      # [128, 128]
_CB_COLS = _AV_OFF + 128
# f32 blob: bh_f32 [128,128] then a64t [64,64] (cols 128..192)
_FB_COLS = 192


def _build_nc():
    fp32 = mybir.dt.float32
    bft = mybir.dt.bfloat16
    Exp = mybir.ActivationFunctionType.Exp
    Ax = mybir.AxisListType
    Alu = mybir.AluOpType

    nc = bass.Bass()
    xt_ext = nc.declare_dram_parameter("xt", [128, H, 2, 128], bft, isOutput=False)
    cb_ext = nc.declare_dram_parameter("cblob", [128, _CB_COLS], bft, isOutput=False)
    fb_ext = nc.declare_dram_parameter("fblob", [128, _FB_COLS], fp32, isOutput=False)
    y_ext = nc.declare_dram_parameter("y", [B_LOC, H, W, O], bft, isOutput=True)
    # HBM bounce for the h<->w shuffle: layout (h, b, w, d) -- strided write,
    # contiguous read, pipelined in 8h-eighths (no global barrier)
    tbounce = nc.dram_tensor("tbounce", [H, B_LOC, W, D], bft)

    with tile.TileContext(nc) as tc:
        with (
            tc.tile_pool(name="const", bufs=1) as cpool,
            tc.tile_pool(name="ek", bufs=4) as ek_pool,
            tc.tile_pool(name="slab", bufs=1) as slab_pool,
            tc.tile_pool(name="ot", bufs=4) as ot_pool,
            tc.tile_pool(name="y", bufs=4) as y_pool,
        ):
            # ---- constants; wkv first (first matmul's dependency)
            cblob = cpool.tile([128, _CB_COLS], bft)
            fblob = cpool.tile([128, _FB_COLS], fp32)
            xt_slab = slab_pool.tile([128, H, 2, 128], bft)

            # trigger order: wkv -> x piece 0 -> rest of cblob -> staggered x
            # pieces round-robin across Sync/GpSimd (ACT kept free: it is the
            # phase-A pacing engine) -> fblob
            # first 512 cols = wkv cc-chunk 0: everything the first k|v
            # matmul needs besides x piece 0
            nc.sync.dma_start(out=cblob[:, 0:512], in_=cb_ext[:, 0:512])
            # few, staggered x pieces: small head pieces so the first k|v
            # matmul starts early, 8h bodies to stay off the semaphore pool
            xp = [(0, 2), (2, 4), (4, 8)] + [(q * 8, (q + 1) * 8)
                                             for q in range(1, 8)]

            def _xpiece(eng, q):
                a, b = xp[q]
                eng.dma_start(out=xt_slab[:, a:b], in_=xt_ext[:, a:b])

            _xpiece(nc.sync, 0)
            _xpiece(nc.gpsimd, 1)
            _xpiece(nc.gpsimd, 2)
            nc.sync.dma_start(out=cblob[:, 512:], in_=cb_ext[:, 512:])
            for q in range(3, 10):
                eng = (nc.sync, nc.gpsimd)[q % 2]
                _xpiece(eng, q)
            nc.sync.dma_start(out=fblob[:], in_=fb_ext[:])

            wkv_sb = cblob[:, _WKV_OFF:_WKV_OFF + 1024].rearrange(
                "p (c d) -> p c d", c=2)
            wp_sb = cblob[:, _WP_OFF:_WP_OFF + 512].rearrange(
                "p (c d) -> p c d", c=2)
            bh_bd = cblob[:, _BH_OFF:_BH_OFF + 128]
            av_mix = cblob[:, _AV_OFF:_AV_OFF + 128]
            bh_f32 = fblob[:, 0:128]
            a64t = fblob[0:64, 128:192]

            u_slab = slab_pool.tile([128, H, D], bft)      # [(b,w), h, d]
            t_slab = slab_pool.tile([128, H, D], bft)      # [(b,w), h, d]
            tt_slab = slab_pool.tile([128, W, D], bft)     # [(h,b)=2h+b, w, d]
            ek_slab = slab_pool.tile([128, H, D], bft)     # [(b,w), h, d]
            s_slab = slab_pool.tile([128, H], fp32)        # [(b,w), h]
            sl1 = slab_pool.tile([128, 2, 8, 128], fp32)   # s-tree lvl1 (2 bufs)
            sl2 = slab_pool.tile([128, 2, 8, 64], fp32)    # s-tree lvl2 (2 bufs)
            u_flat = u_slab.rearrange("p h d -> p (h d)")
            t_flat = t_slab.rearrange("p h d -> p (h d)")
            tt_flat = tt_slab.rearrange("p w d -> p (w d)")
            tb_w = tbounce.rearrange("h b w d -> b w h d")   # leg-1 write view
            tb_r = tbounce.rearrange("h b w d -> (h b) w d")  # leg-2 read view
            sth = cpool.tile([64, 128], fp32, tag="sth")

            with (
                tc.tile_pool(name="ps_kv", bufs=3, space="PSUM") as ps_kv_pool,
                tc.tile_pool(name="ps_t", bufs=2, space="PSUM") as ps_t_pool,
            ):
                # ---- phase A: k|v matmuls + batched exp + u-mul; s via a
                # 2-level GpSimd add-tree (SBUF-only, so the PSUM-blind
                # engine finally carries real work) + cheap DVE finisher;
                # horizontal pass and the h<->w shuffle interleaved per 8h
                for hp in range(H // 2):          # h-pair index
                    h0 = 2 * hp
                    ps2 = ps_kv_pool.tile([128, 2, 512], fp32)
                    for j in range(2):
                        nc.tensor.matmul(ps2[:, j, :], xt_slab[:, h0 + j, 0, :],
                                         wkv_sb[:, 0, :], start=True, stop=False)
                        nc.tensor.matmul(ps2[:, j, :], xt_slab[:, h0 + j, 1, :],
                                         wkv_sb[:, 1, :], start=False, stop=True)
                    ek = ek_slab[:, h0:h0 + 2, :]
                    nc.scalar.activation(ek, ps2[:, :, 0:256], Exp)
                    nc.vector.tensor_mul(u_slab[:, h0:h0 + 2, :], ek,
                                         ps2[:, :, 256:512])
                    # horizontal matmul + drain PER PAIR: its only dep is
                    # this pair's u, so the drain never waits on late PE
                    # work from a whole 8h batch (that coupling made the
                    # batched variants latency-bound)
                    ps_t = ps_t_pool.tile([128, 512], fp32)
                    nc.tensor.matmul(ps_t[:], bh_bd[:],
                                     u_flat[:, hp * 512:(hp + 1) * 512],
                                     start=True, stop=True)
                    if hp % 8 in (0, 2, 4, 6, 1):
                        nc.scalar.copy(t_flat[:, hp * 512:(hp + 1) * 512],
                                       ps_t[:])
                    else:
                        nc.vector.tensor_copy(
                            t_flat[:, hp * 512:(hp + 1) * 512], ps_t[:])

                    # bounce, all on Sync HWDGE (RTL descriptor generation;
                    # SWDGE's software loop costs ~3-5us per strided leg).
                    # Leg 2 of block q is emitted one BLOCK later so its
                    # leg-1-receipt wait is already satisfied when it
                    # reaches the head of the FIFO -- Sync never stalls and
                    # nothing piles up at the end of phase A.  The last
                    # block bounces at PAIR granularity so the final chain
                    # is one small leg1+leg2.
                    if hp < 28 and hp % 4 == 3:
                        q = hp // 4      # 0..6
                        h8 = 8 * q
                        nc.sync.dma_start(
                            out=tb_w[:, :, h8:h8 + 8, :],
                            in_=t_slab[:, h8:h8 + 8, :])
                        if q >= 1:
                            for wl, wh in ((0, 32), (32, 64)):
                                nc.sync.dma_start(
                                    out=tt_slab[16 * q - 16:16 * q, wl:wh, :],
                                    in_=tb_r[16 * q - 16:16 * q, wl:wh, :])
                    elif hp >= 28:
                        # q7: per-pair leg 1 right after this pair's drain
                        nc.sync.dma_start(
                            out=tb_w[:, :, h0:h0 + 2, :],
                            in_=t_slab[:, h0:h0 + 2, :])
                        if hp == 29:
                            # q6's leg 2 rides here (its leg 1 receipt is
                            # ~a pair period old by now)
                            for wl, wh in ((0, 32), (32, 64)):
                                nc.sync.dma_start(
                                    out=tt_slab[96:112, wl:wh, :],
                                    in_=tb_r[96:112, wl:wh, :])
                        elif hp >= 30:
                            # q7 leg 2 for the pair whose leg 1 landed two
                            # pairs ago
                            p = hp - 2
                            nc.sync.dma_start(
                                out=tt_slab[4 * p:4 * p + 4, :, :],
                                in_=tb_r[4 * p:4 * p + 4, :, :])

                    if hp % 4 == 3:
                        q = hp // 4
                        h8 = 8 * q
                        qb = q % 2
                        # s-tree for this 8h: two GpSimd adds halving d; the
                        # DVE finisher is deferred one block so it never
                        # heads DVE's FIFO while the tree is still running
                        ekq = ek_slab[:, h8:h8 + 8, :]
                        nc.gpsimd.tensor_add(sl1[:, qb], ekq[:, :, 0:128],
                                             ekq[:, :, 128:256])
                        nc.gpsimd.tensor_add(sl2[:, qb], sl1[:, qb, :, 0:64],
                                             sl1[:, qb, :, 64:128])
                        if q >= 1:
                            nc.vector.tensor_reduce(
                                s_slab[:, h8 - 8:h8], sl2[:, 1 - qb],
                                Ax.X, Alu.add)
                # last block's s finisher + the last two pairs' leg 2s
                nc.vector.tensor_reduce(s_slab[:, H - 8:H], sl2[:, 1],
                                        Ax.X, Alu.add)
                for p in (30, 31):
                    nc.sync.dma_start(
                        out=tt_slab[4 * p:4 * p + 4, :, :],
                        in_=tb_r[4 * p:4 * p + 4, :, :])

                # norm-horizontal (swapped: out comes transposed [h, (b,w)])
                ps_sh = ps_t_pool.tile([64, 128], fp32, tag="ps_t")
                nc.tensor.matmul(ps_sh[:], s_slab[:], bh_f32[:],
                                 start=True, stop=True)
                nc.vector.tensor_copy(sth[:], ps_sh[:])

            with (
                tc.tile_pool(name="ps_g", bufs=4, space="PSUM") as ps_g_pool,
                tc.tile_pool(name="ps_y", bufs=4, space="PSUM") as ps_y_pool,
            ):
                # norm-vertical: per-b matmuls into partition halves (col tiling)
                ps_n = ps_y_pool.tile([128, 64], fp32, tag="ps_y")
                for b in range(B_LOC):
                    nc.tensor.matmul(ps_n[b * 64:(b + 1) * 64, :], a64t[:],
                                     sth[:, b * 64:(b + 1) * 64],
                                     start=True, stop=True,
                                     tile_position=(0, b * 64))
                ntmp = cpool.tile([128, 64], fp32, tag="ntmp")
                nc.vector.tensor_scalar_add(ntmp[:], ps_n[:], 1e-8)
                rnorm = cpool.tile([128, 64], fp32, tag="rnorm")
                nc.vector.reciprocal(rnorm[:], ntmp[:])

                # ---- phase B: vertical pass (swapped -> feature-major) + proj
                for g in range(W // 2):           # w-pair per psum group
                    ps_g = ps_g_pool.tile([128, 512], fp32)
                    for q in range(4):
                        cch = g * 4 + q   # chunk = (w = cch>>1, dhalf = cch&1)
                        nc.tensor.matmul(ps_g[:, q * 128:(q + 1) * 128],
                                         tt_flat[:, cch * 128:(cch + 1) * 128],
                                         av_mix[:], start=True, stop=True)
                    ot = ot_pool.tile([128, 512], bft)
                    if g % 2 == 0:
                        nc.scalar.copy(ot[:], ps_g[:])
                    else:
                        nc.vector.tensor_copy(ot[:], ps_g[:])

                    for wi in range(2):
                        w = 2 * g + wi
                        base = wi * 256
                        ps_y = ps_y_pool.tile([128, O], fp32)
                        nc.tensor.matmul(ps_y[:], ot[:, base:base + 128],
                                         wp_sb[:, 0, :], start=True, stop=False)
                        nc.tensor.matmul(ps_y[:], ot[:, base + 128:base + 256],
                                         wp_sb[:, 1, :], start=False, stop=True)
                        if w % 4 == 0:
                            y4 = y_pool.tile([128, 4, O], bft)
                        yt = y4[:, w % 4, :]
                        # y drains: half DVE, half ACT (paired with the
                        # opposite engine's ot drain for this g)
                        if wi == (g % 2):
                            nc.scalar.mul(yt[:], ps_y[:], rnorm[:, w:w + 1])
                        else:
                            nc.vector.tensor_scalar_mul(yt[:], ps_y[:],
                                                        rnorm[:, w:w + 1])
                        if w % 4 == 3:
                            w0 = w - 3
                            nc.sync.dma_start(
                                out=y_ext[:, :, w0:w0 + 4, :].rearrange(
                                    "b h w o -> (b h) w o"),
                                in_=y4[:])

    _split_multi_waits(nc)
    return nc


def _host_prep(x, w_h, w_v, Wk, Wv, Wp):
    """Build per-core input maps (all layout/packing on host, compute on device)."""
    A = np.zeros((H, H), np.float32)
    Bm = np.zeros((W, W), np.float32)
    for h in range(H):
        for hp in range(max(0, h - R), min(H, h + R + 1)):
            A[h, hp] = np.exp(w_h[hp - h + R] * ((hp - h) - h))
    for w in range(W):
        for wp in range(max(0, w - R), min(W, w + R + 1)):
            Bm[w, wp] = np.exp(w_v[wp - w + R] * ((wp - w) - w))

    eye2 = np.eye(2, dtype=np.float32)
    bh_bd = np.kron(eye2, Bm.T)                      # lhsT for horizontal
    # vertical rhs, rows (h',b)-interleaved, cols (b,h) b-major:
    av_mix = np.zeros((128, 128), np.float32)
    for b in range(B_LOC):
        for h in range(H):
            for hp in range(max(0, h - R), min(H, h + R + 1)):
                av_mix[2 * hp + b, 64 * b + h] = A[h, hp]

    # wkv[ci, cc, j] = Wk[j, cc*128+ci] (j<256) else Wv[j-256, ...]
    wkv = np.concatenate([Wk.T, Wv.T], axis=1)       # [C, 2D]
    wkv = wkv.reshape(2, 128, 2 * D).transpose(1, 0, 2)      # [ci, cc, 512]
    wp = Wp.T.reshape(2, 128, O).transpose(1, 0, 2)          # [di, dc, o]

    cblob = np.empty((128, _CB_COLS), np.float32)
    cblob[:, _WKV_OFF:_WKV_OFF + 1024] = wkv.reshape(128, 1024)
    cblob[:, _WP_OFF:_WP_OFF + 512] = wp.reshape(128, 512)
    cblob[:, _BH_OFF:_BH_OFF + 128] = bh_bd
    cblob[:, _AV_OFF:_AV_OFF + 128] = av_mix
    cblob = cblob.astype(bf16)

    fblob = np.zeros((128, _FB_COLS), np.float32)
    fblob[:, 0:128] = bh_bd
    fblob[0:64, 128:192] = A.T

    in_maps = []
    for c in range(N_CORES):
        xl = x[c * B_LOC:(c + 1) * B_LOC]            # (2, 64, 64, 256)
        t = xl.reshape(B_LOC, H, W, 2, 128)          # b h w cc ci
        xt = np.ascontiguousarray(
            t.transpose(4, 1, 3, 0, 2).reshape(128, H, 2, 128)).astype(bf16)
        in_maps.append({"xt": xt, "cblob": cblob, "fblob": fblob})
    return in_maps


def kernel(x, w_h, w_v, Wk, Wv, Wp):
    global LAST_RESULT, _CACHED_NC
    x = np.asarray(x, dtype=np.float32)
    w_h = np.asarray(w_h, dtype=np.float32)
    w_v = np.asarray(w_v, dtype=np.float32)
    Wk = np.asarray(Wk, dtype=np.float32)
    Wv = np.asarray(Wv, dtype=np.float32)
    Wp = np.asarray(Wp, dtype=np.float32)

    in_maps = _host_prep(x, w_h, w_v, Wk, Wv, Wp)
    if _CACHED_NC is None:
        _CACHED_NC = _build_nc()
    res = run_bass_kernel_spmd(_CACHED_NC, in_maps, core_ids=list(range(N_CORES)))
    LAST_RESULT = res

    out = np.empty((B_FULL, H, W, O), np.float32)
    for c in range(N_CORES):
        yc = np.asarray(res.results[c]["y"]).astype(np.float32)
        out[c * B_LOC:(c + 1) * B_LOC] = yc
    return out



# revision 29
# speedup vs baseline: 1.2074x; 1.0828x over previous
"""AFT2D attention Trainium2 kernel (8 NeuronCores, data-parallel over batch).

Math: the reference's 5x5 windowed attention with positional bias
    wgt = exp(w_h[ii]*(di-h) + w_v[jj]*(dj-w) + k[h+di, w+dj]) * mask
factorizes exactly: exp(bias) separates into per-row and per-column factors,
so with ek = exp(k), u = ek*v, s = sum_d ek:
    out  = A @ (B ∘w u)      (two banded 64x64 matrix passes, h then w)
    norm = A @ (B ∘w s)
    y    = (out @ Wp^T) / (norm + eps)      (normalize commutes past Wp)
where A[h,h'] = exp(w_h[h'-h+R]*((h'-h)-h)) on the band, B likewise over w.

v3 pipeline (b_loc=2 images; partitions = (b,w), then (h,b) after the
h<->w shuffle). Engine budget per 8h in phase A ~4.1-4.3us on each of
PE/ACT/DVE/GpSimd; phase B balanced ACT/DVE under the PE's ~0.86us/g:
  1. k|v matmul per h-pair into a 2-bank psum tile [128, 2, 512]
     (x-tiles as PE stationary, streaming [Wk^T | Wv^T]).
  2. ACT: ONE batched exp per h-pair [128, 2, 256] -> ek.
  3. DVE: u = ek * v per h-pair.  GpSimd: s = sum_d ek per h via
     tensor_scalar+accum_out (reads ek from SBUF; GpSimd has no PSUM
     port so it can't take drains).
  4. Horizontal pass per 8h: 4 matmuls vs blkdiag(B^T,B^T); drains
     alternate ACT/DVE.
  5. h<->w shuffle via HBM bounce (big batched transfers; SBUF->SBUF
     per-h DMAs lose on descriptor dispatch): BOTH legs on Sync HWDGE,
     leg 2 directly behind leg 1 in the FIFO (its sem wait overlaps the
     next block's compute; Sync dispatch is far snappier than GpSimd
     SWDGE which used to add ~5us to the A->B transition).
  6. Vertical pass, swapped operands -> feature-major psum [d-half, (b,h)]
     = the projection's lhsT; projection matmul; ot drains alternate
     ACT/DVE; y drains (scaled by 1/norm) split 50/50 ACT/DVE.
Input x DMAs are triggered round-robin from Sync/GpSimd (ACT stays free:
it paces phase A) so the first k|v matmul starts early.
Norm path runs in f32 on the side (tiny matmuls; per-b col-tiled output).
"""
import sys

sys.path.insert(0, "/opt/trn_rl_repo")

import numpy as np
import ml_dtypes

import concourse.bass as bass
import concourse.mybir as mybir
import concourse.tile as tile
from concourse.bass_utils import run_bass_kernel_spmd

bf16 = ml_dtypes.bfloat16

N_CORES = 8
B_FULL, H, W, C = 16, 64, 64, 256
D = 256   # HID
O = 256   # OUT
R = 2
B_LOC = B_FULL // N_CORES  # 2

LAST_RESULT = None
_CACHED_NC = None


def _split_multi_waits(nc, max_waits=1):
    """This container's walrus accepts at most ONE sync-wait per instruction;
    hoist extras into standalone same-engine no-ops (order-preserving)."""
    n_new = 0
    for func in nc.m.functions:
        for blk in func.blocks:
            new_insts = []
            for inst in blk.instructions:
                si = inst.sync_info
                if si is not None and len(si.on_wait) > max_waits:
                    waits = list(si.on_wait)
                    for w in waits[:-max_waits]:
                        nop = mybir.InstNoOp(
                            name=f"waitsplit-{n_new}-{inst.name}", ins=[], outs=[])
                        nop.engine = inst.engine
                        nop.sync_info = mybir.SyncInfo(on_wait=[w], on_update=[])
                        new_insts.append(nop)
                        n_new += 1
                    si.on_wait = waits[-max_waits:]
                new_insts.append(inst)
            blk.instructions = new_insts
    return n_new


# packed bf16 constant blob column offsets
_WKV_OFF = 0                 # [128, 2, 512]
_WP_OFF = _WKV_OFF + 1024    # [128, 2, 256]
_BH_OFF = _WP_OFF + 512      # [128, 128]
_AV_OFF = _BH_OFF + 128      # [128, 128]
_CB_COLS = _AV_OFF + 128
# f32 blob: bh_f32 [128,128] then a64t [64,64] (cols 128..192)
_FB_COLS = 192


def _build_nc():
    fp32 = mybir.dt.float32
    bft = mybir.dt.bfloat16
    Exp = mybir.ActivationFunctionType.Exp
    Ax = mybir.AxisListType
    Alu = mybir.AluOpType

    nc = bass.Bass()
    xt_ext = nc.declare_dram_parameter("xt", [128, H, 2, 128], bft, isOutput=False)
    cb_ext = nc.declare_dram_parameter("cblob", [128, _CB_COLS], bft, isOutput=False)
    fb_ext = nc.declare_dram_parameter("fblob", [128, _FB_COLS], fp32, isOutput=False)
    y_ext = nc.declare_dram_parameter("y", [B_LOC, H, W, O], bft, isOutput=True)
    # HBM bounce for the h<->w shuffle: layout (h, b, w, d) -- strided write,
    # contiguous read, pipelined in 8h-eighths (no global barrier)
    tbounce = nc.dram_tensor("tbounce", [H, B_LOC, W, D], bft)

    with tile.TileContext(nc) as tc:
        with (
            tc.tile_pool(name="const", bufs=1) as cpool,
            tc.tile_pool(name="ek", bufs=4) as ek_pool,
            tc.tile_pool(name="slab", bufs=1) as slab_pool,
            tc.tile_pool(name="ot", bufs=4) as ot_pool,
            tc.tile_pool(name="y", bufs=4) as y_pool,
        ):
            # ---- constants; wkv first (first matmul's dependency)
            cblob = cpool.tile([128, _CB_COLS], bft)
            fblob = cpool.tile([128, _FB_COLS], fp32)
            xt_slab = slab_pool.tile([128, H, 2, 128], bft)

            # trigger order: wkv -> x piece 0 -> rest of cblob -> staggered x
            # pieces round-robin across Sync/GpSimd (ACT kept free: it is the
            # phase-A pacing engine) -> fblob
            # first 512 cols = wkv cc-chunk 0: everything the first k|v
            # matmul needs besides x piece 0
            nc.sync.dma_start(out=cblob[:, 0:512], in_=cb_ext[:, 0:512])
            # few, staggered x pieces: small head pieces so the first k|v
            # matmul starts early, 8h bodies to stay off the semaphore pool
            xp = [(0, 2), (2, 4), (4, 8)] + [(q * 8, (q + 1) * 8)
                                             for q in range(1, 8)]

            def _xpiece(eng, q):
                a, b = xp[q]
                eng.dma_start(out=xt_slab[:, a:b], in_=xt_ext[:, a:b])

            _xpiece(nc.sync, 0)
            _xpiece(nc.gpsimd, 1)
            _xpiece(nc.gpsimd, 2)
            nc.sync.dma_start(out=cblob[:, 512:], in_=cb_ext[:, 512:])
            for q in range(3, 10):
                eng = (nc.sync, nc.gpsimd)[q % 2]
                _xpiece(eng, q)
            nc.sync.dma_start(out=fblob[:], in_=fb_ext[:])

            wkv_sb = cblob[:, _WKV_OFF:_WKV_OFF + 1024].rearrange(
                "p (c d) -> p c d", c=2)
            wp_sb = cblob[:, _WP_OFF:_WP_OFF + 512].rearrange(
                "p (c d) -> p c d", c=2)
            bh_bd = cblob[:, _BH_OFF:_BH_OFF + 128]
            av_mix = cblob[:, _AV_OFF:_AV_OFF + 128]
            bh_f32 = fblob[:, 0:128]
            a64t = fblob[0:64, 128:192]

            u_slab = slab_pool.tile([128, H, D], bft)      # [(b,w), h, d]
            t_slab = slab_pool.tile([128, H, D], bft)      # [(b,w), h, d]
            tt_slab = slab_pool.tile([128, W, D], bft)     # [(h,b)=2h+b, w, d]
            ek_slab = slab_pool.tile([128, H, D], bft)     # [(b,w), h, d]
            s_slab = slab_pool.tile([128, H], fp32)        # [(b,w), h]
            sl1 = slab_pool.tile([128, 2, 8, 128], fp32)   # s-tree lvl1 (2 bufs)
            sl2 = slab_pool.tile([128, 2, 8, 64], fp32)    # s-tree lvl2 (2 bufs)
            u_flat = u_slab.rearrange("p h d -> p (h d)")
            t_flat = t_slab.rearrange("p h d -> p (h d)")
            tt_flat = tt_slab.rearrange("p w d -> p (w d)")
            tb_w = tbounce.rearrange("h b w d -> b w h d")   # leg-1 write view
            tb_r = tbounce.rearrange("h b w d -> (h b) w d")  # leg-2 read view
            sth = cpool.tile([64, 128], fp32, tag="sth")

            with (
                tc.tile_pool(name="ps_kv", bufs=3, space="PSUM") as ps_kv_pool,
                tc.tile_pool(name="ps_t", bufs=2, space="PSUM") as ps_t_pool,
            ):
                # ---- phase A: k|v matmuls + batched exp + u-mul; s via a
                # 2-level GpSimd add-tree (SBUF-only, so the PSUM-blind
                # engine finally carries real work) + cheap DVE finisher;
                # horizontal pass and the h<->w shuffle interleaved per 8h
                for hp in range(H // 2):          # h-pair index
                    h0 = 2 * hp
                    ps2 = ps_kv_pool.tile([128, 2, 512], fp32)
                    for j in range(2):
                        nc.tensor.matmul(ps2[:, j, :], xt_slab[:, h0 + j, 0, :],
                                         wkv_sb[:, 0, :], start=True, stop=False)
                        nc.tensor.matmul(ps2[:, j, :], xt_slab[:, h0 + j, 1, :],
                                         wkv_sb[:, 1, :], start=False, stop=True)
                    ek = ek_slab[:, h0:h0 + 2, :]
                    nc.scalar.activation(ek, ps2[:, :, 0:256], Exp)
                    nc.vector.tensor_mul(u_slab[:, h0:h0 + 2, :], ek,
                                         ps2[:, :, 256:512])
                    # horizontal matmul + drain PER PAIR: its only dep is
                    # this pair's u, so the drain never waits on late PE
                    # work from a whole 8h batch (that coupling made the
                    # batched variants latency-bound)
                    ps_t = ps_t_pool.tile([128, 512], fp32)
                    nc.tensor.matmul(ps_t[:], bh_bd[:],
                                     u_flat[:, hp * 512:(hp + 1) * 512],
                                     start=True, stop=True)
                    if hp % 8 in (0, 2, 4, 6, 1):
                        nc.scalar.copy(t_flat[:, hp * 512:(hp + 1) * 512],
                                       ps_t[:])
                    else:
                        nc.vector.tensor_copy(
                            t_flat[:, hp * 512:(hp + 1) * 512], ps_t[:])

                    # bounce, all on Sync HWDGE (RTL descriptor generation;
                    # SWDGE's software loop costs ~3-5us per strided leg).
                    # Leg 1 per PAIR: the writes spread evenly through
                    # phase A instead of bunching per 8h.  Leg 2 of block q
                    # is emitted two BLOCKS later: HBM write receipts run
                    # ~5us here, so one block of deferral was not enough to
                    # pre-satisfy the wait -- two is, and the Sync FIFO
                    # then never stalls.
                    nc.sync.dma_start(
                        out=tb_w[:, :, h0:h0 + 2, :],
                        in_=t_slab[:, h0:h0 + 2, :])
                    if hp % 4 == 3 and hp >= 11:
                        qd = hp // 4 - 2     # 0..5
                        for wl, wh in ((0, 32), (32, 64)):
                            nc.sync.dma_start(
                                out=tt_slab[16 * qd:16 * qd + 16, wl:wh, :],
                                in_=tb_r[16 * qd:16 * qd + 16, wl:wh, :])

                    if hp % 4 == 3:
                        q = hp // 4
                        h8 = 8 * q
                        qb = q % 2
                        # s-tree for this 8h: two GpSimd adds halving d; the
                        # DVE finisher is deferred one block so it never
                        # heads DVE's FIFO while the tree is still running
                        ekq = ek_slab[:, h8:h8 + 8, :]
                        nc.gpsimd.tensor_add(sl1[:, qb], ekq[:, :, 0:128],
                                             ekq[:, :, 128:256])
                        nc.gpsimd.tensor_add(sl2[:, qb], sl1[:, qb, :, 0:64],
                                             sl1[:, qb, :, 64:128])
                        if q >= 1:
                            nc.vector.tensor_reduce(
                                s_slab[:, h8 - 8:h8], sl2[:, 1 - qb],
                                Ax.X, Alu.add)
                # last block's s finisher + the last two blocks' leg 2s
                # (low-w halves first: phase B walks w ascending)
                nc.vector.tensor_reduce(s_slab[:, H - 8:H], sl2[:, 1],
                                        Ax.X, Alu.add)
                for wl, wh in ((0, 32), (32, 64)):
                    for qd in (6, 7):
                        nc.sync.dma_start(
                            out=tt_slab[16 * qd:16 * qd + 16, wl:wh, :],
                            in_=tb_r[16 * qd:16 * qd + 16, wl:wh, :])

                # norm-horizontal (swapped: out comes transposed [h, (b,w)])
                ps_sh = ps_t_pool.tile([64, 128], fp32, tag="ps_t")
                nc.tensor.matmul(ps_sh[:], s_slab[:], bh_f32[:],
                                 start=True, stop=True)
                nc.vector.tensor_copy(sth[:], ps_sh[:])

            with (
                tc.tile_pool(name="ps_g", bufs=4, space="PSUM") as ps_g_pool,
                tc.tile_pool(name="ps_y", bufs=4, space="PSUM") as ps_y_pool,
            ):
                # norm-vertical: per-b matmuls into partition halves (col tiling)
                ps_n = ps_y_pool.tile([128, 64], fp32, tag="ps_y")
                for b in range(B_LOC):
                    nc.tensor.matmul(ps_n[b * 64:(b + 1) * 64, :], a64t[:],
                                     sth[:, b * 64:(b + 1) * 64],
                                     start=True, stop=True,
                                     tile_position=(0, b * 64))
                ntmp = cpool.tile([128, 64], fp32, tag="ntmp")
                nc.vector.tensor_scalar_add(ntmp[:], ps_n[:], 1e-8)
                rnorm = cpool.tile([128, 64], fp32, tag="rnorm")
                nc.vector.reciprocal(rnorm[:], ntmp[:])

                # ---- phase B: vertical pass (swapped -> feature-major) + proj
                for g in range(W // 2):           # w-pair per psum group
                    ps_g = ps_g_pool.tile([128, 512], fp32)
                    for q in range(4):
                        cch = g * 4 + q   # chunk = (w = cch>>1, dhalf = cch&1)
                        nc.tensor.matmul(ps_g[:, q * 128:(q + 1) * 128],
                                         tt_flat[:, cch * 128:(cch + 1) * 128],
                                         av_mix[:], start=True, stop=True)
                    ot = ot_pool.tile([128, 512], bft)
                    if g % 2 == 0:
                        nc.scalar.copy(ot[:], ps_g[:])
                    else:
                        nc.vector.tensor_copy(ot[:], ps_g[:])

                    for wi in range(2):
                        w = 2 * g + wi
                        base = wi * 256
                        ps_y = ps_y_pool.tile([128, O], fp32)
                        nc.tensor.matmul(ps_y[:], ot[:, base:base + 128],
                                         wp_sb[:, 0, :], start=True, stop=False)
                        nc.tensor.matmul(ps_y[:], ot[:, base + 128:base + 256],
                                         wp_sb[:, 1, :], start=False, stop=True)
                        if w % 4 == 0:
                            y4 = y_pool.tile([128, 4, O], bft)
                        yt = y4[:, w % 4, :]
                        # y drains: half DVE, half ACT (paired with the
                        # opposite engine's ot drain for this g)
                        if wi == (g % 2):
                            nc.scalar.mul(yt[:], ps_y[:], rnorm[:, w:w + 1])
                        else:
                            nc.vector.tensor_scalar_mul(yt[:], ps_y[:],
                                                        rnorm[:, w:w + 1])
                        if w % 4 == 3:
                            w0 = w - 3
                            nc.sync.dma_start(
                                out=y_ext[:, :, w0:w0 + 4, :].rearrange(
                                    "b h w o -> (b h) w o"),
                                in_=y4[:])

    _split_multi_waits(nc)
    return nc


def _host_prep(x, w_h, w_v, Wk, Wv, Wp):
    """Build per-core input maps (all layout/packing on host, compute on device)."""
    A = np.zeros((H, H), np.float32)
    Bm = np.zeros((W, W), np.float32)
    for h in range(H):
        for hp in range(max(0, h - R), min(H, h + R + 1)):
            A[h, hp] = np.exp(w_h[hp - h + R] * ((hp - h) - h))
    for w in range(W):
        for wp in range(max(0, w - R), min(W, w + R + 1)):
            Bm[w, wp] = np.exp(w_v[wp - w + R] * ((wp - w) - w))

    eye2 = np.eye(2, dtype=np.float32)
    bh_bd = np.kron(eye2, Bm.T)                      # lhsT for horizontal
    # vertical rhs, rows (h',b)-interleaved, cols (b,h) b-major:
    av_mix = np.zeros((128, 128), np.float32)
    for b in range(B_LOC):
        for h in range(H):
            for hp in range(max(0, h - R), min(H, h + R + 1)):
                av_mix[2 * hp + b, 64 * b + h] = A[h, hp]

    # wkv[ci, cc, j] = Wk[j, cc*128+ci] (j<256) else Wv[j-256, ...]
    wkv = np.concatenate([Wk.T, Wv.T], axis=1)       # [C, 2D]
    wkv = wkv.reshape(2, 128, 2 * D).transpose(1, 0, 2)      # [ci, cc, 512]
    wp = Wp.T.reshape(2, 128, O).transpose(1, 0, 2)          # [di, dc, o]

    cblob = np.empty((128, _CB_COLS), np.float32)
    cblob[:, _WKV_OFF:_WKV_OFF + 1024] = wkv.reshape(128, 1024)
    cblob[:, _WP_OFF:_WP_OFF + 512] = wp.reshape(128, 512)
    cblob[:, _BH_OFF:_BH_OFF + 128] = bh_bd
    cblob[:, _AV_OFF:_AV_OFF + 128] = av_mix
    cblob = cblob.astype(bf16)

    fblob = np.zeros((128, _FB_COLS), np.float32)
    fblob[:, 0:128] = bh_bd
    fblob[0:64, 128:192] = A.T

    in_maps = []
    for c in range(N_CORES):
        xl = x[c * B_LOC:(c + 1) * B_LOC]            # (2, 64, 64, 256)
        t = xl.reshape(B_LOC, H, W, 2, 128)          # b h w cc ci
        xt = np.ascontiguousarray(
            t.transpose(4, 1, 3, 0, 2).reshape(128, H, 2, 128)).astype(bf16)
        in_maps.append({"xt": xt, "cblob": cblob, "fblob": fblob})
    return in_maps


def kernel(x, w_h, w_v, Wk, Wv, Wp):
    global LAST_RESULT, _CACHED_NC
    x = np.asarray(x, dtype=np.float32)
    w_h = np.asarray(w_h, dtype=np.float32)
    w_v = np.asarray(w_v, dtype=np.float32)
    Wk = np.asarray(Wk, dtype=np.float32)
    Wv = np.asarray(Wv, dtype=np.float32)
    Wp = np.asarray(Wp, dtype=np.float32)

    in_maps = _host_prep(x, w_h, w_v, Wk, Wv, Wp)
    if _CACHED_NC is None:
        _CACHED_NC = _build_nc()
    res = run_bass_kernel_spmd(_CACHED_NC, in_maps, core_ids=list(range(N_CORES)))
    LAST_RESULT = res

    out = np.empty((B_FULL, H, W, O), np.float32)
    for c in range(N_CORES):
        yc = np.asarray(res.results[c]["y"]).astype(np.float32)
        out[c * B_LOC:(c + 1) * B_LOC] = yc
    return out

